# revision 29
# baseline (speedup 1.0000x reference)
"""CLIP attention (B=2, S=2048, H=768, 12 heads) on 8 trn2 NeuronCores.

Sharding: data-parallel over batch (2) x tensor-parallel over head groups
(4 groups of 3 heads).  Each core computes, for its (batch, head-group):
    q = x @ Wq_g * 1/sqrt(64) (+ bq_g scaled)      [2048, 192]
    k = x @ Wk_g                                    [2048, 192]
    v = x @ Wv_g                                    [2048, 192]
    per head: P' = exp(q k^T)   (no max subtraction; logits are O(1))
              O'^T, s via ones-augmented V:  o_ps = [V_h | 1]^T @ P'^T
    y = sum_h (O'_h / s_h) @ Wo_h                   [2048, 768]
Host sums the 4 head-group partials per batch and adds the exactly-folded
bias terms (bk drops out of softmax; bv/bo fold to a constant row).

Matmul convention: nc.tensor.matmul(out, lhsT, rhs) => out = lhsT.T @ rhs,
contraction over the partition dim of both operands.

v3 scheduling/layout:
  - x arrives host-pre-transposed [C, S]; few LARGE prologue DMAs split
    across both HWDGE queues (many small ones serialize on the recycled
    DMA-completion semaphores).
  - heads 0 and 1 live in ONE qT01/kT01 tile (partition halves) exactly as
    the projection psum produces them -> one DVE copy per piece, no
    duplication, and the two K=64 logits matmuls pair ACROSS heads in
    disjoint PE row groups.  Head 2 keeps the duplicated-rows layout and
    pairs with itself across query sub-blocks.
  - head-2 q and k are projected TOGETHER (lhsT = [Wq_h2 | Wk_h2], M=128).
  - q bias added during the psum->sbuf copy (per-partition scalar).
  - attention passes: P01(qb) covers heads 0+1 for a 512-query block; H2(qh)
    covers head 2 for a 1024-query half.  Pass order interleaves so Y for
    the first half can be dripped mid-kernel.  Each pass yields two
    [65, 512] o_ps tiles (V'-augmented: row 64 = softmax denominators).
  - per o_ps unit: fast DVE extraction (s row -> bf16, raw O' -> bf16)
    frees PSUM; the broadcast (K=1 outer-product matmul) + reciprocal +
    scale chain is DRIPPED into the next pass so it never blocks the
    in-order PE queue.  Result: PRE-SCALED oTs tiles.
  - Y projection: heads 0+1 as one K=128 matmul (stacked oTs01), head 2
    K=64; all heads accumulate in one PSUM tile per token block; no vector
    scaling of Y.  Y for token blocks of the first half is dripped into
    later passes; the second half's h0+h1 partials are dripped into the
    last pass so the tail only runs head-2 matmul + add + DMA.
  - output DMAs split across the sync/scalar queues, issued per block.
"""

import sys

if "/opt/trn_rl_repo" not in sys.path:
    sys.path.insert(0, "/opt/trn_rl_repo")

import numpy as np
import ml_dtypes

import concourse.bacc as bacc
import concourse.tile as tile
from concourse import mybir
from concourse.bass_utils import run_bass_kernel_spmd

BF16 = mybir.dt.bfloat16
F32 = mybir.dt.float32
MULT = mybir.AluOpType.mult

S = 2048          # sequence length
C = 768           # hidden
HD = 64           # head dim
NCORES = 8
GROUPS = 4        # head groups (tensor parallel)
HPG = 3           # heads per group
GF = HPG * HD     # group feature width = 192
NCC = C // 128    # contraction chunks = 6
NQB = S // 128    # token blocks = 16
NKB = S // 128    # key blocks = 16
QH = S // 2       # queries per half = 1024


def build_program():
    nc = bacc.Bacc("TRN2", target_bir_lowering=False, debug=False)

    xt_dram = nc.dram_tensor("x", (C, S), BF16, kind="ExternalInput").ap()
    wq = nc.dram_tensor("wq", (C, 128), BF16, kind="ExternalInput").ap()
    wk = nc.dram_tensor("wk", (C, 128), BF16, kind="ExternalInput").ap()
    wqk2 = nc.dram_tensor("wqk2", (C, 128), BF16, kind="ExternalInput").ap()
    wv = nc.dram_tensor("wv", (C, GF), BF16, kind="ExternalInput").ap()
    wo01 = nc.dram_tensor("wo01", (128, C), BF16, kind="ExternalInput").ap()
    wo2 = nc.dram_tensor("wo2", (128, C), BF16, kind="ExternalInput").ap()
    bqc = nc.dram_tensor("bqc", (128, 2), F32, kind="ExternalInput").ap()
    out = nc.dram_tensor("out", (S, C), F32, kind="ExternalOutput").ap()
    import os
    DBG = os.environ.get("KDBG") == "1"
    if DBG:
        dbg_oTs01 = nc.dram_tensor("dbg_oTs01", (2, 128, QH), BF16,
                                   kind="ExternalOutput").ap()
        dbg_oTs2 = nc.dram_tensor("dbg_oTs2", (2, HD, QH), BF16,
                                  kind="ExternalOutput").ap()
        dbg_sbf = nc.dram_tensor("dbg_sbf", (HPG, 1, S), F32,
                                 kind="ExternalOutput").ap()
        dbg_qT01 = nc.dram_tensor("dbg_qT01", (128, S), BF16,
                                  kind="ExternalOutput").ap()
        dbg_kT01 = nc.dram_tensor("dbg_kT01", (128, S), BF16,
                                  kind="ExternalOutput").ap()
        dbg_qT2 = nc.dram_tensor("dbg_qT2", (128, S), BF16,
                                 kind="ExternalOutput").ap()
        dbg_kT2 = nc.dram_tensor("dbg_kT2", (128, S), BF16,
                                 kind="ExternalOutput").ap()
        dbg_vS = nc.dram_tensor("dbg_vS", (NKB, 128, HPG * (HD + 1)), BF16,
                                kind="ExternalOutput").ap()
        dbg_sb = nc.dram_tensor("dbg_sb", (2, HD, 512), F32,
                                kind="ExternalOutput").ap()
        dbg_rb = nc.dram_tensor("dbg_rb", (2, HD, 512), F32,
                                kind="ExternalOutput").ap()
        dbg_oraw = nc.dram_tensor("dbg_oraw", (2, HD, 512), BF16,
                                  kind="ExternalOutput").ap()

    with tile.TileContext(nc) as tc:
        with tc.tile_pool(name="consts", bufs=1) as consts:
            wq_sb = consts.tile([128, NCC, 128], BF16)
            wk_sb = consts.tile([128, NCC, 128], BF16)
            wqk2_sb = consts.tile([128, NCC, 128], BF16)
            wv_sb = consts.tile([128, NCC, GF], BF16)
            wo01_sb = consts.tile([128, C], BF16)
            wo2_sb = consts.tile([128, C], BF16)
            bq_sb = consts.tile([128, 2], F32)
            xT_sb = consts.tile([128, NCC, S], BF16)
            xT = [xT_sb[:, c, :] for c in range(NCC)]
            # prologue: few large DMAs, both queues, consumption order
            nc.sync.dma_start(out=xT_sb[:, 0, :], in_=xt_dram[0:128, :])
            nc.scalar.dma_start(
                out=wq_sb[:], in_=wq.rearrange("(c p) f -> p c f", p=128)
            )
            nc.sync.dma_start(out=bq_sb[:], in_=bqc[:])
            nc.scalar.dma_start(out=xT_sb[:, 1, :], in_=xt_dram[128:256, :])
            nc.sync.dma_start(out=xT_sb[:, 2, :], in_=xt_dram[256:384, :])
            nc.scalar.dma_start(out=xT_sb[:, 3, :], in_=xt_dram[384:512, :])
            nc.sync.dma_start(out=xT_sb[:, 4, :], in_=xt_dram[512:640, :])
            nc.scalar.dma_start(out=xT_sb[:, 5, :], in_=xt_dram[640:768, :])
            nc.sync.dma_start(
                out=wv_sb[:], in_=wv.rearrange("(c p) f -> p c f", p=128)
            )
            nc.scalar.dma_start(
                out=wk_sb[:], in_=wk.rearrange("(c p) f -> p c f", p=128)
            )
            nc.sync.dma_start(out=wo01_sb[:], in_=wo01[:])
            nc.scalar.dma_start(
                out=wqk2_sb[:], in_=wqk2.rearrange("(c p) f -> p c f", p=128)
            )
            nc.sync.dma_start(out=wo2_sb[:], in_=wo2[:])

            ones_col = consts.tile([1, HD], BF16)
            nc.vector.memset(ones_col[:], 1.0)

            # heads 0,1 combined (h0 rows 0-63, h1 rows 64-127)
            qT01 = consts.tile([128, S], BF16, name="qT01")
            kT01 = consts.tile([128, S], BF16, name="kT01")
            # head 2, rows duplicated for row-group pairing
            qT2 = consts.tile([128, S], BF16, name="qT2")
            kT2 = consts.tile([128, S], BF16, name="kT2")
            vS = [consts.tile([128, HPG, HD + 1], BF16, name=f"vS{t}")
                  for t in range(NKB)]
            # PRE-SCALED attention outputs (h0 rows 0-63, h1 rows 64-127)
            oTs01 = [consts.tile([128, QH], BF16, name=f"oTs01_{qh}")
                     for qh in range(2)]
            oTs2 = [consts.tile([128, QH], BF16, name=f"oTs2_{qh}")
                    for qh in range(2)]
            s_bf = [consts.tile([1, S], F32, name=f"s_bf{h}")
                    for h in range(HPG)]
            ys = [consts.tile([128, C], F32, name=f"ys{t}") for t in range(NQB)]

            def qk_piece_body(pool, tag, w_sb, n, kind):
                ps = pool.tile([128, 512], F32, tag=tag, name=f"{kind}{n}")
                for c in range(NCC):
                    nc.tensor.matmul(
                        ps[:],
                        w_sb[:, c, :],
                        xT[c][:, n * 512 : (n + 1) * 512],
                        start=(c == 0),
                        stop=(c == NCC - 1),
                    )
                ns = slice(n * 512, (n + 1) * 512)
                if kind == "q":      # one combined copy + bias
                    nc.vector.tensor_scalar_add(
                        qT01[:, ns], ps[:], bq_sb[:, 0:1]
                    )
                elif kind == "k":
                    nc.vector.tensor_copy(kT01[:, ns], ps[:])
                else:  # 'qk2': q2 rows 0-63 (+bias, dup), k2 rows 64-127 (dup)
                    nc.vector.tensor_scalar_add(
                        qT2[0:64, ns], ps[0:64, :], bq_sb[0:64, 1:2]
                    )
                    nc.vector.tensor_scalar_add(
                        qT2[64:128, ns], ps[0:64, :], bq_sb[64:128, 1:2]
                    )
                    nc.vector.tensor_copy(kT2[0:64, ns], ps[64:128, :])
                    nc.vector.tensor_copy(kT2[64:128, ns], ps[64:128, :])

            def v_piece_body(pool, tag, t):
                vps = pool.tile([128, GF], F32, tag=tag, name=f"vp{t}")
                for c in range(NCC):
                    nc.tensor.matmul(
                        vps[:],
                        xT[c][:, t * 128 : (t + 1) * 128],
                        wv_sb[:, c, :],
                        start=(c == 0),
                        stop=(c == NCC - 1),
                    )
                nc.vector.tensor_copy(
                    vS[t][:, :, 0:HD],
                    vps[:].rearrange("p (h d) -> p h d", h=HPG),
                )
                nc.vector.memset(vS[t][:, :, HD : HD + 1], 1.0)

            # ------------- phase A: just enough to start attention -------------
            with tc.tile_pool(name="pp", bufs=3, space="PSUM") as pp, \
                 tc.tile_pool(name="vpp", bufs=2, space="PSUM") as vpp:
                # q0/k0 interleaved by contraction chunk: the PE pipelines
                # across the xT chunk arrivals instead of stalling per piece
                ps_q = pp.tile([128, 512], F32, tag="pp", name="q0")
                ps_k = pp.tile([128, 512], F32, tag="pp", name="k0")
                for c in range(NCC):
                    nc.tensor.matmul(ps_q[:], wq_sb[:, c, :], xT[c][:, 0:512],
                                     start=(c == 0), stop=(c == NCC - 1))
                    nc.tensor.matmul(ps_k[:], wk_sb[:, c, :], xT[c][:, 0:512],
                                     start=(c == 0), stop=(c == NCC - 1))
                nc.vector.tensor_scalar_add(qT01[:, 0:512], ps_q[:],
                                            bq_sb[:, 0:1])
                nc.vector.tensor_copy(kT01[:, 0:512], ps_k[:])
                v_piece_body(vpp, "vps", 0)
                v_piece_body(vpp, "vps", 1)

            # ---------------- attention passes ----------------
            with tc.tile_pool(name="flex", bufs=1, space="PSUM") as flex, \
                 tc.tile_pool(name="lt_ps", bufs=2, space="PSUM") as ltp, \
                 tc.tile_pool(name="o_ps", bufs=2, space="PSUM") as opp, \
                 tc.tile_pool(name="att_sb", bufs=4) as asb:

                def task_v(t):
                    def run():
                        v_piece_body(flex, "flex", t)
                    return run

                def task_qk(w_sb, n, kind):
                    def run():
                        qk_piece_body(flex, "flex", w_sb, n, kind)
                    return run

                ext_oraw = {}

                # deferred part of the softmax-scale chain (dripped into the
                # NEXT pass so the outer-product matmuls never stall the PE)
                def ext_finish(h, qa):
                    def run():
                        sb = asb.tile([HD, 512], F32, tag="sbc",
                                      name=f"sb{h}_{qa}")
                        nc.gpsimd.partition_broadcast(
                            sb[:], s_bf[h][:, qa : qa + 512], channels=HD
                        )
                        rb = asb.tile([HD, 512], F32, tag="rb",
                                      name=f"rb{h}_{qa}")
                        nc.vector.reciprocal_approx_fast(out=rb[:], in_=sb[:])
                        oraw = ext_oraw.pop((h, qa))
                        oraw = oraw if not hasattr(oraw, 'space') else oraw
                        cs = slice(qa % QH, (qa % QH) + 512)
                        if h < 2:
                            dsts = [oTs01[qa // QH][h * 64 : (h + 1) * 64, cs]]
                        else:
                            dsts = [oTs2[qa // QH][0:64, cs],
                                    oTs2[qa // QH][64:128, cs]]
                        for dst in dsts:
                            nc.vector.tensor_tensor(out=dst, in0=oraw[:],
                                                    in1=rb[:], op=MULT)
                        if DBG and h == 2 and qa < 1024:
                            j = qa // 512
                            nc.sync.dma_start(out=dbg_sb[j], in_=sb[:])
                            nc.sync.dma_start(out=dbg_rb[j], in_=rb[:])
                            nc.sync.dma_start(out=dbg_oraw[j], in_=oraw[:])
                    return run

                def extract(o_ps, h, qa, keep_ops=False):
                    """Immediate extraction: frees o_ps (s row + raw O').
                    keep_ops: let the scale chain read o_ps directly (last
                    pass only — nothing reuses the PSUM after it)."""
                    nc.vector.tensor_copy(s_bf[h][:, qa : qa + 512],
                                          o_ps[HD : HD + 1, :])
                    if keep_ops:
                        ext_oraw[(h, qa)] = o_ps[0:HD, :]
                        return
                    oraw = asb.tile([HD, 512], BF16, tag="oraw",
                                    name=f"oraw{h}_{qa}")
                    nc.vector.tensor_copy(oraw[:], o_ps[0:HD, :])
                    ext_oraw[(h, qa)] = oraw

                def task_y_full(qh, t):
                    def run():
                        yp = flex.tile([128, C], F32, tag="flex",
                                       name=f"y{qh}_{t}")
                        tbs = slice((t % 8) * 128, (t % 8) * 128 + 128)
                        nc.tensor.matmul(yp[:, 0:512], oTs01[qh][:, tbs],
                                         wo01_sb[:, 0:512],
                                         start=True, stop=False)
                        nc.tensor.matmul(yp[:, 512:C], oTs01[qh][:, tbs],
                                         wo01_sb[:, 512:C],
                                         start=True, stop=False)
                        nc.tensor.matmul(yp[:, 0:512], oTs2[qh][0:64, tbs],
                                         wo2_sb[0:64, 0:512],
                                         start=False, stop=True)
                        nc.tensor.matmul(yp[:, 512:C], oTs2[qh][64:128, tbs],
                                         wo2_sb[64:128, 512:C],
                                         start=False, stop=True)
                        nc.vector.tensor_copy(ys[t][:], yp[:])
                        eng = nc.sync if t % 2 == 0 else nc.scalar
                        eng.dma_start(out=out[t * 128 : (t + 1) * 128, :],
                                      in_=ys[t][:])
                    return run

                def task_y01(qh, t):
                    def run():
                        yp = flex.tile([128, C], F32, tag="flex",
                                       name=f"y01_{t}")
                        tbs = slice((t % 8) * 128, (t % 8) * 128 + 128)
                        nc.tensor.matmul(yp[:, 0:512], oTs01[qh][:, tbs],
                                         wo01_sb[:, 0:512],
                                         start=True, stop=True)
                        nc.tensor.matmul(yp[:, 512:C], oTs01[qh][:, tbs],
                                         wo01_sb[:, 512:C],
                                         start=True, stop=True)
                        nc.vector.tensor_copy(ys[t][:], yp[:])
                    return run

                def attn_pass(spec, drip, keep_ops=False):
                    """spec = (klo, qlo_t, qlo, khi, qhi_t, qhi, uA, uB);
                    lo/hi are the two PE row groups; each unit (h, qa)
                    accumulates a [65, 512] o_ps over all key blocks."""
                    (klo, qlo_t, qlo, khi, qhi_t, qhi, uA, uB) = spec
                    oA = opp.tile([HD + 1, 512], F32, tag="o",
                                  name=f"oA{uA[0]}_{uA[1]}")
                    oB = opp.tile([HD + 1, 512], F32, tag="o",
                                  name=f"oB{uB[0]}_{uB[1]}")
                    for kb in range(NKB):
                        kbs = slice(kb * 128, (kb + 1) * 128)
                        lt = ltp.tile([128, 1024], F32, tag="lt")
                        nc.tensor.matmul(
                            lt[:, 0:512], klo[0:64, kbs],
                            qlo_t[0:64, qlo : qlo + 512],
                            start=True, stop=True,
                        )
                        nc.tensor.matmul(
                            lt[:, 512:1024], khi[64:128, kbs],
                            qhi_t[64:128, qhi : qhi + 512],
                            start=True, stop=True,
                        )
                        elt = asb.tile([128, 1024], BF16, tag="elt")
                        nc.scalar.activation(
                            elt[:], lt[:], mybir.ActivationFunctionType.Exp
                        )
                        nc.tensor.matmul(
                            oA[:], vS[kb][:, uA[0], :], elt[:, 0:512],
                            start=(kb == 0), stop=(kb == NKB - 1),
                        )
                        nc.tensor.matmul(
                            oB[:], vS[kb][:, uB[0], :], elt[:, 512:1024],
                            start=(kb == 0), stop=(kb == NKB - 1),
                        )
                        for tsk in drip.get(kb, ()):
                            tsk()
                    extract(oA, *uA, keep_ops=keep_ops)
                    extract(oB, *uB, keep_ops=keep_ops)
                    return uA, uB

                # pass specs ------------------------------------------------
                def p01(qb):
                    qa = qb * 512
                    return (kT01, qT01, qa, kT01, qT01, qa,
                            (0, qa), (1, qa))

                def h2(qh):
                    qa = qh * QH
                    return (kT2, qT2, qa, kT2, qT2, qa + 512,
                            (2, qa), (2, qa + 512))

                def sched(pairs):
                    d = {}
                    for kb, ts in pairs:
                        d.setdefault(kb, []).append(ts)
                    return d

                passes = [p01(0), p01(1), h2(0), p01(2), p01(3), h2(1)]
                extra = {
                    0: [(0, task_v(2)), (0, task_v(3)), (1, task_v(4)),
                        (2, task_qk(wk_sb, 1, "k")), (3, task_v(5)),
                        (4, task_v(6)), (5, task_qk(wk_sb, 2, "k")),
                        (5, task_v(7)), (6, task_v(8)), (7, task_v(9)),
                        (8, task_qk(wk_sb, 3, "k")), (8, task_v(10)),
                        (9, task_v(11)), (10, task_v(12)), (11, task_v(13)),
                        (12, task_v(14)), (13, task_v(15)),
                        (14, task_qk(wq_sb, 1, "q"))],
                    1: [(5, task_qk(wq_sb, 2, "q")),
                        (8, task_qk(wqk2_sb, 0, "qk2")),
                        (11, task_qk(wqk2_sb, 1, "qk2"))],
                    2: [(4, task_qk(wqk2_sb, 2, "qk2")),
                        (9, task_qk(wqk2_sb, 3, "qk2"))],
                    3: [(4, task_y_full(0, 0)), (6, task_y_full(0, 1)),
                        (8, task_y_full(0, 2)), (10, task_y_full(0, 3)),
                        (6, task_qk(wq_sb, 3, "q"))],
                    4: [(4, task_y_full(0, 4)), (6, task_y_full(0, 5)),
                        (8, task_y_full(0, 6)), (10, task_y_full(0, 7))],
                    5: [(2, task_y01(1, 8)), (3, task_y01(1, 9)),
                        (5, task_y01(1, 10)), (6, task_y01(1, 11)),
                        (8, task_y01(1, 12)), (9, task_y01(1, 13)),
                        (10, task_y01(1, 14)), (11, task_y01(1, 15))],
                }

                for i, spec in enumerate(passes):
                    last = i == len(passes) - 1
                    uA, uB = attn_pass(spec, sched(list(extra.get(i, ()))),
                                       keep_ops=last)
                    ext_finish(*uA)()
                    ext_finish(*uB)()

                # tail: head-2 Y + out
                for t in range(8, 16):
                    tbs = slice((t - 8) * 128, (t - 8) * 128 + 128)
                    pool_t = flex if t % 2 == 0 else ltp
                    yp = pool_t.tile([128, C], F32,
                                     tag="flex" if t % 2 == 0 else "lt",
                                     name=f"y2_{t}")
                    nc.tensor.matmul(yp[:, 0:512], oTs2[1][0:64, tbs],
                                     wo2_sb[0:64, 0:512],
                                     start=True, stop=True)
                    nc.tensor.matmul(yp[:, 512:C], oTs2[1][64:128, tbs],
                                     wo2_sb[64:128, 512:C],
                                     start=True, stop=True)
                    nc.vector.tensor_add(ys[t][:], ys[t][:], yp[:])
                    eng = nc.sync if t % 2 == 0 else nc.scalar
                    eng.dma_start(out=out[t * 128 : (t + 1) * 128, :],
                                  in_=ys[t][:])

            if DBG:
                for qh in range(2):
                    nc.sync.dma_start(out=dbg_oTs01[qh], in_=oTs01[qh][:])
                    nc.sync.dma_start(out=dbg_oTs2[qh], in_=oTs2[qh][:])
                for h in range(HPG):
                    nc.sync.dma_start(out=dbg_sbf[h], in_=s_bf[h][:])
                nc.sync.dma_start(out=dbg_qT01[:], in_=qT01[:])
                nc.sync.dma_start(out=dbg_kT01[:], in_=kT01[:])
                nc.sync.dma_start(out=dbg_qT2[:], in_=qT2[:])
                nc.sync.dma_start(out=dbg_kT2[:], in_=kT2[:])
                for t in range(NKB):
                    nc.sync.dma_start(
                        out=dbg_vS[t],
                        in_=vS[t][:].rearrange("p h d -> p (h d)"),
                    )

    nc.compile()
    return nc


_COMPILED_NC = None


def _get_nc():
    global _COMPILED_NC
    if _COMPILED_NC is None:
        _COMPILED_NC = build_program()
    return _COMPILED_NC


def make_in_maps(x, Wq, bq, Wk, bk, Wv, bv, Wo, bo):
    scale = 1.0 / np.sqrt(HD)
    bf = ml_dtypes.bfloat16
    # host-side pre-transpose: kernel takes x^T [C, S]
    x_bf = [np.ascontiguousarray(x[b].T).astype(bf) for b in range(x.shape[0])]
    Wq = np.asarray(Wq)
    Wk = np.asarray(Wk)
    Wv = np.asarray(Wv)
    Wo = np.asarray(Wo)
    bq = np.asarray(bq)
    in_maps = []
    for cid in range(NCORES):
        b, g = divmod(cid, GROUPS)
        cols = slice(g * GF, (g + 1) * GF)
        wq_g = Wq[:, cols] * scale
        wk_g = Wk[:, cols]
        wqk2 = np.concatenate([wq_g[:, 128:192], wk_g[:, 128:192]], axis=1)
        bq_g = bq[cols] * scale
        bqc = np.empty((128, 2), dtype=np.float32)
        bqc[:, 0] = bq_g[0:128]            # [bq_h0 | bq_h1]
        bqc[0:64, 1] = bq_g[128:192]       # bq_h2 duplicated
        bqc[64:128, 1] = bq_g[128:192]
        in_maps.append(
            {
                "x": x_bf[b],
                "wq": np.ascontiguousarray(wq_g[:, 0:128]).astype(bf),
                "wk": np.ascontiguousarray(wk_g[:, 0:128]).astype(bf),
                "wqk2": np.ascontiguousarray(wqk2).astype(bf),
                "wv": np.ascontiguousarray(Wv[:, cols]).astype(bf),
                "wo01": np.ascontiguousarray(Wo[cols, :][0:128, :]).astype(bf),
                "wo2": np.ascontiguousarray(np.concatenate([Wo[cols, :][128:192, :]] * 2, axis=0)).astype(bf),
                "bqc": bqc,
            }
        )
    return in_maps


def gather_output(results, x, Wv, bv, Wo, bo):
    B = x.shape[0]
    out = np.zeros((B, S, C), dtype=np.float32)
    for cid in range(NCORES):
        b, _ = divmod(cid, GROUPS)
        out[b] += results[cid]["out"]
    # exact bias folds: bk cancels in softmax; v-bias -> bv @ Wo; + bo
    out += (np.asarray(bv, np.float32) @ np.asarray(Wo, np.float32)
            + np.asarray(bo, np.float32))
    return out


def kernel(x, Wq, bq, Wk, bk, Wv, bv, Wo, bo):
    x = np.asarray(x)
    nc = _get_nc()
    in_maps = make_in_maps(x, Wq, bq, Wk, bk, Wv, bv, Wo, bo)
    res = run_bass_kernel_spmd(nc, in_maps, core_ids=list(range(NCORES)))
    return gather_output(res.results, x, Wv, bv, Wo, bo)


# revision 31
# speedup vs baseline: 1.0018x; 1.0018x over previous
"""CLIP attention (B=2, S=2048, H=768, 12 heads) on 8 trn2 NeuronCores.

Sharding: data-parallel over batch (2) x tensor-parallel over head groups
(4 groups of 3 heads).  Each core computes, for its (batch, head-group):
    q = x @ Wq_g * 1/sqrt(64) (+ bq_g scaled)      [2048, 192]
    k = x @ Wk_g                                    [2048, 192]
    v = x @ Wv_g                                    [2048, 192]
    per head: P' = exp(q k^T)   (no max subtraction; logits are O(1))
              O'^T, s via ones-augmented V:  o_ps = [V_h | 1]^T @ P'^T
    y = sum_h (O'_h / s_h) @ Wo_h                   [2048, 768]
Host sums the 4 head-group partials per batch and adds the exactly-folded
bias terms (bk drops out of softmax; bv/bo fold to a constant row).

Matmul convention: nc.tensor.matmul(out, lhsT, rhs) => out = lhsT.T @ rhs,
contraction over the partition dim of both operands.

v3 scheduling/layout:
  - x arrives host-pre-transposed [C, S]; few LARGE prologue DMAs split
    across both HWDGE queues (many small ones serialize on the recycled
    DMA-completion semaphores).
  - heads 0 and 1 live in ONE qT01/kT01 tile (partition halves) exactly as
    the projection psum produces them -> one DVE copy per piece, no
    duplication, and the two K=64 logits matmuls pair ACROSS heads in
    disjoint PE row groups.  Head 2 keeps the duplicated-rows layout and
    pairs with itself across query sub-blocks.
  - head-2 q and k are projected TOGETHER (lhsT = [Wq_h2 | Wk_h2], M=128).
  - q bias added during the psum->sbuf copy (per-partition scalar).
  - attention passes: P01(qb) covers heads 0+1 for a 512-query block; H2(qh)
    covers head 2 for a 1024-query half.  Pass order interleaves so Y for
    the first half can be dripped mid-kernel.  Each pass yields two
    [65, 512] o_ps tiles (V'-augmented: row 64 = softmax denominators).
  - per o_ps unit: fast DVE extraction (s row -> bf16, raw O' -> bf16)
    frees PSUM; the broadcast (K=1 outer-product matmul) + reciprocal +
    scale chain is DRIPPED into the next pass so it never blocks the
    in-order PE queue.  Result: PRE-SCALED oTs tiles.
  - Y projection: heads 0+1 as one K=128 matmul (stacked oTs01), head 2
    K=64; all heads accumulate in one PSUM tile per token block; no vector
    scaling of Y.  Y for token blocks of the first half is dripped into
    later passes; the second half's h0+h1 partials are dripped into the
    last pass so the tail only runs head-2 matmul + add + DMA.
  - output DMAs split across the sync/scalar queues, issued per block.
"""

import sys

if "/opt/trn_rl_repo" not in sys.path:
    sys.path.insert(0, "/opt/trn_rl_repo")

import numpy as np
import ml_dtypes

import concourse.bacc as bacc
import concourse.tile as tile
from concourse import mybir
from concourse.bass_utils import run_bass_kernel_spmd

BF16 = mybir.dt.bfloat16
F32 = mybir.dt.float32
MULT = mybir.AluOpType.mult

S = 2048          # sequence length
C = 768           # hidden
HD = 64           # head dim
NCORES = 8
GROUPS = 4        # head groups (tensor parallel)
HPG = 3           # heads per group
GF = HPG * HD     # group feature width = 192
NCC = C // 128    # contraction chunks = 6
NQB = S // 128    # token blocks = 16
NKB = S // 128    # key blocks = 16
QH = S // 2       # queries per half = 1024


def build_program():
    nc = bacc.Bacc("TRN2", target_bir_lowering=False, debug=False)

    xt_dram = nc.dram_tensor("x", (C, S), BF16, kind="ExternalInput").ap()
    wq = nc.dram_tensor("wq", (C, 128), BF16, kind="ExternalInput").ap()
    wk = nc.dram_tensor("wk", (C, 128), BF16, kind="ExternalInput").ap()
    wqk2 = nc.dram_tensor("wqk2", (C, 128), BF16, kind="ExternalInput").ap()
    wv = nc.dram_tensor("wv", (C, GF), BF16, kind="ExternalInput").ap()
    wo01 = nc.dram_tensor("wo01", (128, C), BF16, kind="ExternalInput").ap()
    wo2 = nc.dram_tensor("wo2", (128, C), BF16, kind="ExternalInput").ap()
    bqc = nc.dram_tensor("bqc", (128, 2), F32, kind="ExternalInput").ap()
    out = nc.dram_tensor("out", (S, C), F32, kind="ExternalOutput").ap()
    import os
    DBG = os.environ.get("KDBG") == "1"
    if DBG:
        dbg_oTs01 = nc.dram_tensor("dbg_oTs01", (2, 128, QH), BF16,
                                   kind="ExternalOutput").ap()
        dbg_oTs2 = nc.dram_tensor("dbg_oTs2", (2, HD, QH), BF16,
                                  kind="ExternalOutput").ap()
        dbg_sbf = nc.dram_tensor("dbg_sbf", (HPG, 1, S), F32,
                                 kind="ExternalOutput").ap()
        dbg_qT01 = nc.dram_tensor("dbg_qT01", (128, S), BF16,
                                  kind="ExternalOutput").ap()
        dbg_kT01 = nc.dram_tensor("dbg_kT01", (128, S), BF16,
                                  kind="ExternalOutput").ap()
        dbg_qT2 = nc.dram_tensor("dbg_qT2", (128, S), BF16,
                                 kind="ExternalOutput").ap()
        dbg_kT2 = nc.dram_tensor("dbg_kT2", (128, S), BF16,
                                 kind="ExternalOutput").ap()
        dbg_vS = nc.dram_tensor("dbg_vS", (NKB, 128, HPG * (HD + 1)), BF16,
                                kind="ExternalOutput").ap()
        dbg_sb = nc.dram_tensor("dbg_sb", (2, HD, 512), F32,
                                kind="ExternalOutput").ap()
        dbg_rb = nc.dram_tensor("dbg_rb", (2, HD, 512), F32,
                                kind="ExternalOutput").ap()
        dbg_oraw = nc.dram_tensor("dbg_oraw", (2, HD, 512), BF16,
                                  kind="ExternalOutput").ap()

    with tile.TileContext(nc) as tc:
        with tc.tile_pool(name="consts", bufs=1) as consts:
            wq_sb = consts.tile([128, NCC, 128], BF16)
            wk_sb = consts.tile([128, NCC, 128], BF16)
            wqk2_sb = consts.tile([128, NCC, 128], BF16)
            wv_sb = consts.tile([128, NCC, GF], BF16)
            wo01_sb = consts.tile([128, C], BF16)
            wo2_sb = consts.tile([128, C], BF16)
            bq_sb = consts.tile([128, 2], F32)
            xT_sb = consts.tile([128, NCC, S], BF16)
            xT = [xT_sb[:, c, :] for c in range(NCC)]
            # prologue: few large DMAs, both queues, consumption order
            nc.sync.dma_start(out=xT_sb[:, 0, :], in_=xt_dram[0:128, :])
            nc.scalar.dma_start(
                out=wq_sb[:], in_=wq.rearrange("(c p) f -> p c f", p=128)
            )
            nc.gpsimd.dma_start(out=bq_sb[:], in_=bqc[:])
            nc.scalar.dma_start(out=xT_sb[:, 1, :], in_=xt_dram[128:256, :])
            nc.sync.dma_start(out=xT_sb[:, 2, :], in_=xt_dram[256:384, :])
            nc.scalar.dma_start(out=xT_sb[:, 3, :], in_=xt_dram[384:512, :])
            nc.sync.dma_start(out=xT_sb[:, 4, :], in_=xt_dram[512:640, :])
            nc.scalar.dma_start(out=xT_sb[:, 5, :], in_=xt_dram[640:768, :])
            nc.gpsimd.dma_start(
                out=wv_sb[:], in_=wv.rearrange("(c p) f -> p c f", p=128)
            )
            nc.scalar.dma_start(
                out=wk_sb[:], in_=wk.rearrange("(c p) f -> p c f", p=128)
            )
            nc.gpsimd.dma_start(out=wo01_sb[:], in_=wo01[:])
            nc.gpsimd.dma_start(
                out=wqk2_sb[:], in_=wqk2.rearrange("(c p) f -> p c f", p=128)
            )
            nc.gpsimd.dma_start(out=wo2_sb[:], in_=wo2[:])

            ones_col = consts.tile([1, HD], BF16)
            nc.vector.memset(ones_col[:], 1.0)

            # heads 0,1 combined (h0 rows 0-63, h1 rows 64-127)
            qT01 = consts.tile([128, S], BF16, name="qT01")
            kT01 = consts.tile([128, S], BF16, name="kT01")
            # head 2, rows duplicated for row-group pairing
            qT2 = consts.tile([128, S], BF16, name="qT2")
            kT2 = consts.tile([128, S], BF16, name="kT2")
            vS = [consts.tile([128, HPG, HD + 1], BF16, name=f"vS{t}")
                  for t in range(NKB)]
            # PRE-SCALED attention outputs (h0 rows 0-63, h1 rows 64-127)
            oTs01 = [consts.tile([128, QH], BF16, name=f"oTs01_{qh}")
                     for qh in range(2)]
            oTs2 = [consts.tile([128, QH], BF16, name=f"oTs2_{qh}")
                    for qh in range(2)]
            s_bf = [consts.tile([1, S], F32, name=f"s_bf{h}")
                    for h in range(HPG)]
            ys = [consts.tile([128, C], F32, name=f"ys{t}") for t in range(NQB)]

            def qk_piece_body(pool, tag, w_sb, n, kind):
                ps = pool.tile([128, 512], F32, tag=tag, name=f"{kind}{n}")
                for c in range(NCC):
                    nc.tensor.matmul(
                        ps[:],
                        w_sb[:, c, :],
                        xT[c][:, n * 512 : (n + 1) * 512],
                        start=(c == 0),
                        stop=(c == NCC - 1),
                    )
                ns = slice(n * 512, (n + 1) * 512)
                if kind == "q":      # one combined copy + bias
                    nc.vector.tensor_scalar_add(
                        qT01[:, ns], ps[:], bq_sb[:, 0:1]
                    )
                elif kind == "k":
                    nc.vector.tensor_copy(kT01[:, ns], ps[:])
                else:  # 'qk2': q2 rows 0-63 (+bias, dup), k2 rows 64-127 (dup)
                    nc.vector.tensor_scalar_add(
                        qT2[0:64, ns], ps[0:64, :], bq_sb[0:64, 1:2]
                    )
                    nc.vector.tensor_scalar_add(
                        qT2[64:128, ns], ps[0:64, :], bq_sb[64:128, 1:2]
                    )
                    nc.vector.tensor_copy(kT2[0:64, ns], ps[64:128, :])
                    nc.vector.tensor_copy(kT2[64:128, ns], ps[64:128, :])

            def v_piece_body(pool, tag, t):
                vps = pool.tile([128, GF], F32, tag=tag, name=f"vp{t}")
                for c in range(NCC):
                    nc.tensor.matmul(
                        vps[:],
                        xT[c][:, t * 128 : (t + 1) * 128],
                        wv_sb[:, c, :],
                        start=(c == 0),
                        stop=(c == NCC - 1),
                    )
                nc.vector.tensor_copy(
                    vS[t][:, :, 0:HD],
                    vps[:].rearrange("p (h d) -> p h d", h=HPG),
                )
                nc.vector.memset(vS[t][:, :, HD : HD + 1], 1.0)

            # ------------- phase A: just enough to start attention -------------
            with tc.tile_pool(name="pp", bufs=3, space="PSUM") as pp, \
                 tc.tile_pool(name="vpp", bufs=2, space="PSUM") as vpp:
                # q0/k0 interleaved by contraction chunk: the PE pipelines
                # across the xT chunk arrivals instead of stalling per piece
                ps_q = pp.tile([128, 512], F32, tag="pp", name="q0")
                ps_k = pp.tile([128, 512], F32, tag="pp", name="k0")
                for c in range(NCC):
                    nc.tensor.matmul(ps_q[:], wq_sb[:, c, :], xT[c][:, 0:512],
                                     start=(c == 0), stop=(c == NCC - 1))
                    nc.tensor.matmul(ps_k[:], wk_sb[:, c, :], xT[c][:, 0:512],
                                     start=(c == 0), stop=(c == NCC - 1))
                nc.vector.tensor_scalar_add(qT01[:, 0:512], ps_q[:],
                                            bq_sb[:, 0:1])
                nc.vector.tensor_copy(kT01[:, 0:512], ps_k[:])
                v_piece_body(vpp, "vps", 0)
                v_piece_body(vpp, "vps", 1)

            # ---------------- attention passes ----------------
            with tc.tile_pool(name="flex", bufs=1, space="PSUM") as flex, \
                 tc.tile_pool(name="lt_ps", bufs=2, space="PSUM") as ltp, \
                 tc.tile_pool(name="o_ps", bufs=2, space="PSUM") as opp, \
                 tc.tile_pool(name="att_sb", bufs=4) as asb:

                def task_v(t):
                    def run():
                        v_piece_body(flex, "flex", t)
                    return run

                def task_qk(w_sb, n, kind):
                    def run():
                        qk_piece_body(flex, "flex", w_sb, n, kind)
                    return run

                ext_oraw = {}

                # deferred part of the softmax-scale chain (dripped into the
                # NEXT pass so the outer-product matmuls never stall the PE)
                def ext_finish(h, qa):
                    def run():
                        sb = asb.tile([HD, 512], F32, tag="sbc",
                                      name=f"sb{h}_{qa}")
                        nc.gpsimd.partition_broadcast(
                            sb[:], s_bf[h][:, qa : qa + 512], channels=HD
                        )
                        rb = asb.tile([HD, 512], F32, tag="rb",
                                      name=f"rb{h}_{qa}")
                        nc.vector.reciprocal_approx_fast(out=rb[:], in_=sb[:])
                        oraw = ext_oraw.pop((h, qa))
                        oraw = oraw if not hasattr(oraw, 'space') else oraw
                        cs = slice(qa % QH, (qa % QH) + 512)
                        if h < 2:
                            dsts = [oTs01[qa // QH][h * 64 : (h + 1) * 64, cs]]
                        else:
                            dsts = [oTs2[qa // QH][0:64, cs],
                                    oTs2[qa // QH][64:128, cs]]
                        for dst in dsts:
                            nc.vector.tensor_tensor(out=dst, in0=oraw[:],
                                                    in1=rb[:], op=MULT)
                        if DBG and h == 2 and qa < 1024:
                            j = qa // 512
                            nc.sync.dma_start(out=dbg_sb[j], in_=sb[:])
                            nc.sync.dma_start(out=dbg_rb[j], in_=rb[:])
                            nc.sync.dma_start(out=dbg_oraw[j], in_=oraw[:])
                    return run

                def extract(o_ps, h, qa, keep_ops=False):
                    """Immediate extraction: frees o_ps (s row + raw O').
                    keep_ops: let the scale chain read o_ps directly (last
                    pass only — nothing reuses the PSUM after it)."""
                    nc.vector.tensor_copy(s_bf[h][:, qa : qa + 512],
                                          o_ps[HD : HD + 1, :])
                    if keep_ops:
                        ext_oraw[(h, qa)] = o_ps[0:HD, :]
                        return
                    oraw = asb.tile([HD, 512], BF16, tag="oraw",
                                    name=f"oraw{h}_{qa}")
                    nc.vector.tensor_copy(oraw[:], o_ps[0:HD, :])
                    ext_oraw[(h, qa)] = oraw

                def task_y_full(qh, t):
                    def run():
                        yp = flex.tile([128, C], F32, tag="flex",
                                       name=f"y{qh}_{t}")
                        tbs = slice((t % 8) * 128, (t % 8) * 128 + 128)
                        nc.tensor.matmul(yp[:, 0:512], oTs01[qh][:, tbs],
                                         wo01_sb[:, 0:512],
                                         start=True, stop=False)
                        nc.tensor.matmul(yp[:, 512:C], oTs01[qh][:, tbs],
                                         wo01_sb[:, 512:C],
                                         start=True, stop=False)
                        nc.tensor.matmul(yp[:, 0:512], oTs2[qh][0:64, tbs],
                                         wo2_sb[0:64, 0:512],
                                         start=False, stop=True)
                        nc.tensor.matmul(yp[:, 512:C], oTs2[qh][64:128, tbs],
                                         wo2_sb[64:128, 512:C],
                                         start=False, stop=True)
                        nc.vector.tensor_copy(ys[t][:], yp[:])
                        # sync queue only: the scalar engine is busy with exp
                        # during these passes (descriptor gen would stall it)
                        nc.sync.dma_start(out=out[t * 128 : (t + 1) * 128, :],
                                          in_=ys[t][:])
                    return run

                def task_y01(qh, t):
                    def run():
                        yp = flex.tile([128, C], F32, tag="flex",
                                       name=f"y01_{t}")
                        tbs = slice((t % 8) * 128, (t % 8) * 128 + 128)
                        nc.tensor.matmul(yp[:, 0:512], oTs01[qh][:, tbs],
                                         wo01_sb[:, 0:512],
                                         start=True, stop=True)
                        nc.tensor.matmul(yp[:, 512:C], oTs01[qh][:, tbs],
                                         wo01_sb[:, 512:C],
                                         start=True, stop=True)
                        nc.vector.tensor_copy(ys[t][:], yp[:])
                    return run

                def attn_pass(spec, drip, keep_ops=False):
                    """spec = (klo, qlo_t, qlo, khi, qhi_t, qhi, uA, uB);
                    lo/hi are the two PE row groups; each unit (h, qa)
                    accumulates a [65, 512] o_ps over all key blocks."""
                    (klo, qlo_t, qlo, khi, qhi_t, qhi, uA, uB) = spec
                    oA = opp.tile([HD + 1, 512], F32, tag="o",
                                  name=f"oA{uA[0]}_{uA[1]}")
                    oB = opp.tile([HD + 1, 512], F32, tag="o",
                                  name=f"oB{uB[0]}_{uB[1]}")
                    for kb in range(NKB):
                        kbs = slice(kb * 128, (kb + 1) * 128)
                        lt = ltp.tile([128, 1024], F32, tag="lt")
                        nc.tensor.matmul(
                            lt[:, 0:512], klo[0:64, kbs],
                            qlo_t[0:64, qlo : qlo + 512],
                            start=True, stop=True,
                        )
                        nc.tensor.matmul(
                            lt[:, 512:1024], khi[64:128, kbs],
                            qhi_t[64:128, qhi : qhi + 512],
                            start=True, stop=True,
                        )
                        elt = asb.tile([128, 1024], BF16, tag="elt")
                        nc.scalar.activation(
                            elt[:], lt[:], mybir.ActivationFunctionType.Exp
                        )
                        nc.tensor.matmul(
                            oA[:], vS[kb][:, uA[0], :], elt[:, 0:512],
                            start=(kb == 0), stop=(kb == NKB - 1),
                        )
                        nc.tensor.matmul(
                            oB[:], vS[kb][:, uB[0], :], elt[:, 512:1024],
                            start=(kb == 0), stop=(kb == NKB - 1),
                        )
                        for tsk in drip.get(kb, ()):
                            tsk()
                    extract(oA, *uA, keep_ops=keep_ops)
                    extract(oB, *uB, keep_ops=keep_ops)
                    return uA, uB

                # pass specs ------------------------------------------------
                def p01(qb):
                    qa = qb * 512
                    return (kT01, qT01, qa, kT01, qT01, qa,
                            (0, qa), (1, qa))

                def h2(qh):
                    qa = qh * QH
                    return (kT2, qT2, qa, kT2, qT2, qa + 512,
                            (2, qa), (2, qa + 512))

                def sched(pairs):
                    d = {}
                    for kb, ts in pairs:
                        d.setdefault(kb, []).append(ts)
                    return d

                passes = [p01(0), p01(1), h2(0), p01(2), p01(3), h2(1)]
                extra = {
                    0: [(0, task_v(2)), (0, task_v(3)), (1, task_v(4)),
                        (2, task_qk(wk_sb, 1, "k")), (3, task_v(5)),
                        (4, task_v(6)), (5, task_qk(wk_sb, 2, "k")),
                        (5, task_v(7)), (6, task_v(8)), (7, task_v(9)),
                        (8, task_qk(wk_sb, 3, "k")), (8, task_v(10)),
                        (9, task_v(11)), (10, task_v(12)), (11, task_v(13)),
                        (12, task_v(14)), (13, task_v(15)),
                        (14, task_qk(wq_sb, 1, "q"))],
                    1: [(5, task_qk(wq_sb, 2, "q")),
                        (8, task_qk(wqk2_sb, 0, "qk2")),
                        (11, task_qk(wqk2_sb, 1, "qk2"))],
                    2: [(4, task_qk(wqk2_sb, 2, "qk2")),
                        (9, task_qk(wqk2_sb, 3, "qk2"))],
                    3: [(4, task_y_full(0, 0)), (6, task_y_full(0, 1)),
                        (8, task_y_full(0, 2)), (10, task_y_full(0, 3)),
                        (6, task_qk(wq_sb, 3, "q"))],
                    4: [(4, task_y_full(0, 4)), (6, task_y_full(0, 5)),
                        (8, task_y_full(0, 6)), (10, task_y_full(0, 7))],
                    5: [(2, task_y01(1, 8)), (3, task_y01(1, 9)),
                        (5, task_y01(1, 10)), (6, task_y01(1, 11)),
                        (8, task_y01(1, 12)), (9, task_y01(1, 13)),
                        (10, task_y01(1, 14)), (11, task_y01(1, 15))],
                }

                for i, spec in enumerate(passes):
                    last = i == len(passes) - 1
                    uA, uB = attn_pass(spec, sched(list(extra.get(i, ()))),
                                       keep_ops=last)
                    ext_finish(*uA)()
                    ext_finish(*uB)()

                # tail: head-2 Y + out
                for t in range(8, 16):
                    tbs = slice((t - 8) * 128, (t - 8) * 128 + 128)
                    pool_t = flex if t % 2 == 0 else ltp
                    yp = pool_t.tile([128, C], F32,
                                     tag="flex" if t % 2 == 0 else "lt",
                                     name=f"y2_{t}")
                    nc.tensor.matmul(yp[:, 0:512], oTs2[1][0:64, tbs],
                                     wo2_sb[0:64, 0:512],
                                     start=True, stop=True)
                    nc.tensor.matmul(yp[:, 512:C], oTs2[1][64:128, tbs],
                                     wo2_sb[64:128, 512:C],
                                     start=True, stop=True)
                    nc.vector.tensor_add(ys[t][:], ys[t][:], yp[:])
                    eng = nc.sync if t % 2 == 0 else nc.scalar
                    eng.dma_start(out=out[t * 128 : (t + 1) * 128, :],
                                  in_=ys[t][:])

            if DBG:
                for qh in range(2):
                    nc.sync.dma_start(out=dbg_oTs01[qh], in_=oTs01[qh][:])
                    nc.sync.dma_start(out=dbg_oTs2[qh], in_=oTs2[qh][:])
                for h in range(HPG):
                    nc.sync.dma_start(out=dbg_sbf[h], in_=s_bf[h][:])
                nc.sync.dma_start(out=dbg_qT01[:], in_=qT01[:])
                nc.sync.dma_start(out=dbg_kT01[:], in_=kT01[:])
                nc.sync.dma_start(out=dbg_qT2[:], in_=qT2[:])
                nc.sync.dma_start(out=dbg_kT2[:], in_=kT2[:])
                for t in range(NKB):
                    nc.sync.dma_start(
                        out=dbg_vS[t],
                        in_=vS[t][:].rearrange("p h d -> p (h d)"),
                    )

    nc.compile()
    return nc


_COMPILED_NC = None


def _get_nc():
    global _COMPILED_NC
    if _COMPILED_NC is None:
        _COMPILED_NC = build_program()
    return _COMPILED_NC


def make_in_maps(x, Wq, bq, Wk, bk, Wv, bv, Wo, bo):
    scale = 1.0 / np.sqrt(HD)
    bf = ml_dtypes.bfloat16
    # host-side pre-transpose: kernel takes x^T [C, S]
    x_bf = [np.ascontiguousarray(x[b].T).astype(bf) for b in range(x.shape[0])]
    Wq = np.asarray(Wq)
    Wk = np.asarray(Wk)
    Wv = np.asarray(Wv)
    Wo = np.asarray(Wo)
    bq = np.asarray(bq)
    in_maps = []
    for cid in range(NCORES):
        b, g = divmod(cid, GROUPS)
        cols = slice(g * GF, (g + 1) * GF)
        wq_g = Wq[:, cols] * scale
        wk_g = Wk[:, cols]
        wqk2 = np.concatenate([wq_g[:, 128:192], wk_g[:, 128:192]], axis=1)
        bq_g = bq[cols] * scale
        bqc = np.empty((128, 2), dtype=np.float32)
        bqc[:, 0] = bq_g[0:128]            # [bq_h0 | bq_h1]
        bqc[0:64, 1] = bq_g[128:192]       # bq_h2 duplicated
        bqc[64:128, 1] = bq_g[128:192]
        in_maps.append(
            {
                "x": x_bf[b],
                "wq": np.ascontiguousarray(wq_g[:, 0:128]).astype(bf),
                "wk": np.ascontiguousarray(wk_g[:, 0:128]).astype(bf),
                "wqk2": np.ascontiguousarray(wqk2).astype(bf),
                "wv": np.ascontiguousarray(Wv[:, cols]).astype(bf),
                "wo01": np.ascontiguousarray(Wo[cols, :][0:128, :]).astype(bf),
                "wo2": np.ascontiguousarray(np.concatenate([Wo[cols, :][128:192, :]] * 2, axis=0)).astype(bf),
                "bqc": bqc,
            }
        )
    return in_maps


def gather_output(results, x, Wv, bv, Wo, bo):
    B = x.shape[0]
    out = np.zeros((B, S, C), dtype=np.float32)
    for cid in range(NCORES):
        b, _ = divmod(cid, GROUPS)
        out[b] += results[cid]["out"]
    # exact bias folds: bk cancels in softmax; v-bias -> bv @ Wo; + bo
    out += (np.asarray(bv, np.float32) @ np.asarray(Wo, np.float32)
            + np.asarray(bo, np.float32))
    return out


def kernel(x, Wq, bq, Wk, bk, Wv, bv, Wo, bo):
    x = np.asarray(x)
    nc = _get_nc()
    in_maps = make_in_maps(x, Wq, bq, Wk, bk, Wv, bv, Wo, bo)
    res = run_bass_kernel_spmd(nc, in_maps, core_ids=list(range(NCORES)))
    return gather_output(res.results, x, Wv, bv, Wo, bo)
